# revision 1
# baseline (speedup 1.0000x reference)
"""3-layer GAT (PyG-style GATConv x3 + global mean pool) on 8 trn2 NeuronCores.

Strategy: nodes are dealt round-robin by descending in-degree to the 8 cores
(dst-sharding).  Per layer, each core runs a dense phase (hW = h @ W plus the
per-node attention logit halves), publishes fp16 "hcat" rows [hW + b | al_s]
which are AllGathered across cores, then an edge phase: for each of its nodes
(128 per chunk, padded slot count K per degree bucket) it gathers the hcat
rows of the slot sources with per-column indirect DMAs, computes
e = exp(leakyrelu(al_s + al_d)), and forms out = (sum_k e * h) / (sum_k e)
with a fold-tree reduction; bias is folded into hcat (softmax weights sum
to 1), relu applied, and the result transposed into SBUF as the next layer's
input.  Layer 3 accumulates a column-sum on PE; the host divides by N and
adds b3.  All per-core programs are identical (SPMD); per-core data differs.
"""
import numpy as np
import concourse.bass as bass
import concourse.bacc as bacc
import concourse.mybir as mybir
import concourse.tile as tile
from concourse.masks import make_identity

P = 128
NEG_SLOPE = 0.2
PAD_ALS = -30000.0  # al_s for padding rows: exp(lrelu(.)) == 0 in fp16
F32 = mybir.dt.float32
F16 = mybir.dt.float16
I32 = mybir.dt.int32
BATCH_SLOTS = 96


class Plan:
    pass


def make_plan(edge_index, N, ncores=8, kstep=2):
    E = edge_index.shape[1]
    src = np.concatenate([edge_index[0].astype(np.int64), np.arange(N, dtype=np.int64)])
    dst = np.concatenate([edge_index[1].astype(np.int64), np.arange(N, dtype=np.int64)])
    deg = np.bincount(dst, minlength=N)
    order = np.argsort(-deg, kind="stable")

    npc = (N + ncores - 1) // ncores
    n_chunks = (npc + P - 1) // P + 1  # last chunk is all-pad
    S = n_chunks * P

    Ks = []
    for j in range(n_chunks):
        g0 = j * P * ncores
        dmax = int(deg[order[g0]]) if g0 < N else 1
        Ks.append(max(kstep, ((dmax + kstep - 1) // kstep) * kstep))
    Ks = np.array(Ks, np.int32)
    TK = int(Ks.sum())

    node_at = np.full((ncores, S), -1, np.int64)
    for c in range(ncores):
        g = np.arange(npc) * ncores + c
        valid = g < N
        node_at[c, :npc][valid] = order[g[valid]]
    row_of = np.zeros(N, np.int64)
    for c in range(ncores):
        m = node_at[c] >= 0
        row_of[node_at[c][m]] = c * S + np.nonzero(m)[0]

    eo = np.argsort(dst, kind="stable")
    src_sorted = src[eo]
    starts = np.zeros(N + 1, np.int64)
    np.cumsum(deg, out=starts[1:])

    idx = np.zeros((ncores, P, TK), np.int32)
    maskD = np.zeros((ncores, P, n_chunks), np.float32)
    off = 0
    for j in range(n_chunks):
        K = int(Ks[j])
        for c in range(ncores):
            block = np.full((P, K), c * S + (S - 1), np.int32)
            nodes = node_at[c, j * P:(j + 1) * P]
            for p in range(P):
                n = nodes[p]
                if n < 0:
                    maskD[c, p, j] = 1.0
                    continue
                s0, s1 = int(starts[n]), int(starts[n + 1])
                block[p, :s1 - s0] = row_of[src_sorted[s0:s1]]
            idx[c, :, off:off + K] = block
        off += K

    iters = []
    j = 0
    off = 0
    while j < n_chunks:
        K = int(Ks[j])
        B = max(1, BATCH_SLOTS // K)
        nb = 1
        while nb < B and j + nb < n_chunks and Ks[j + nb] == K:
            nb += 1
        iters.append((K, j, nb, off))
        off += K * nb
        j += nb

    pl = Plan()
    pl.N, pl.E, pl.ncores = N, E, ncores
    pl.npc, pl.n_chunks, pl.S, pl.TK = npc, n_chunks, S, TK
    pl.Ks, pl.iters, pl.node_at, pl.row_of = Ks, iters, node_at, row_of
    pl.idx, pl.maskD = idx, maskD
    return pl


def layer_inputs(pl, layer, hins, W, a_src, a_dst, b):
    """hins: list of per-core [C0,S] arrays (xT fp32 for layer 0, hT fp16 else)."""
    av = np.stack([np.asarray(a_src).reshape(-1), np.asarray(a_dst).reshape(-1)])
    av = np.tile(av[:, None, :], (1, P, 1)).reshape(2 * P, -1).astype(np.float32)
    Wd = np.asarray(W, np.float32 if layer == 0 else np.float16)
    ins = []
    for c in range(pl.ncores):
        d = {"hin": hins[c], "idx": pl.idx[c], "maskD": pl.maskD[c],
             "W": Wd, "av": av}
        if layer < 2:
            d["bv"] = np.tile(np.asarray(b, np.float32)[None, :], (P, 1))
        ins.append(d)
    return ins


def x_slices(pl, x):
    out = []
    for c in range(pl.ncores):
        xs = np.zeros((pl.S, x.shape[1]), np.float32)
        m = pl.node_at[c] >= 0
        xs[m] = x[pl.node_at[c][m]]
        out.append(np.ascontiguousarray(xs.T))
    return out


def build_program(pl, layer, C0=128, H=(8, 8, 1), CH=(16, 16, 32), ncores=8):
    OC = [H[i] * CH[i] for i in range(3)]
    RL = [((OC[i] + H[i] + 1) // 2) * 2 for i in range(3)]  # 136,136,34
    S, n_chunks, TK = pl.S, pl.n_chunks, pl.TK
    NC = ncores
    MAXB = max(nb for (_, _, nb, _) in pl.iters)
    L = layer

    nc = bacc.Bacc("TRN2", target_bir_lowering=False, debug=False, num_devices=NC)
    t_hin = nc.dram_tensor("hin", [C0, S], F32 if L == 0 else F16,
                           kind="ExternalInput")
    t_idx = nc.dram_tensor("idx", [P, TK], I32, kind="ExternalInput")
    t_maskD = nc.dram_tensor("maskD", [P, n_chunks], F32, kind="ExternalInput")
    t_W = nc.dram_tensor("W", [C0 if L == 0 else OC[0], OC[L]],
                         F32 if L == 0 else F16, kind="ExternalInput")
    t_av_l = nc.dram_tensor("av", [2 * P, OC[L]], F32, kind="ExternalInput")
    t_bv_l = (nc.dram_tensor("bv", [P, OC[L]], F32, kind="ExternalInput")
              if L < 2 else None)
    if L < 2:
        t_hout = nc.dram_tensor("hout", [P, S], F16, kind="ExternalOutput")
    else:
        t_y = nc.dram_tensor("y", [1, OC[2]], F32, kind="ExternalOutput")

    with tile.TileContext(nc) as tc:
        with tc.tile_pool(name="res", bufs=1) as res, \
             tc.tile_pool(name="dram", bufs=1, space="DRAM") as dram, \
             tc.tile_pool(name="dn", bufs=2) as dn, \
             tc.tile_pool(name="dnp", bufs=2, space="PSUM") as dnp, \
             tc.tile_pool(name="eg", bufs=2) as eg, \
             tc.tile_pool(name="eg1", bufs=1) as eg1, \
             tc.tile_pool(name="egp", bufs=2, space="PSUM") as egp:
            hT = res.tile([P, S], F16)
            if L > 0:
                nc.sync.dma_start(hT[:], t_hin.ap())
            alD = res.tile([P, n_chunks * 8], F16)
            mask_sb = res.tile([P, n_chunks], F32)
            nc.sync.dma_start(mask_sb[:], t_maskD.ap())
            ident = res.tile([P, P], F16)
            make_identity(nc, ident[:])
            ones_col = res.tile([P, 1], F16)
            nc.gpsimd.memset(ones_col[:], 1.0)
            W_sb = res.tile(list(t_W.shape), F32 if L == 0 else F16)
            nc.sync.dma_start(W_sb[:], t_W.ap())
            a_src_sb = res.tile([P, OC[L]], F32)
            nc.sync.dma_start(a_src_sb[:], t_av_l.ap()[0:P, :])
            a_dst_sb = res.tile([P, OC[L]], F32)
            nc.sync.dma_start(a_dst_sb[:], t_av_l.ap()[P:2 * P, :])
            av_sb = {L: (a_src_sb, a_dst_sb)}
            bv_sb = {}
            if L < 2:
                bv_t = res.tile([P, OC[L]], F32)
                nc.sync.dma_start(bv_t[:], t_bv_l.ap())
                bv_sb[L] = bv_t

            hcat_loc = {L: dram.tile([S, RL[L]], F16, name="hcl")}
            hcat_full = {L: dram.tile([NC * S, RL[L]], F16, name="hcf",
                                      addr_space="Shared")}

            for l in (L,):
                oc, heads, ch, rl = OC[l], H[l], CH[l], RL[l]
                # ---------------- dense ----------------
                DB = 4
                if True:
                    for it0 in range(0, n_chunks, DB):
                        nb = min(DB, n_chunks - it0)
                        if l == 0:
                            xin = dn.tile([C0, DB * P], F32, tag="xin")
                            nc.sync.dma_start(xin[:, :nb * P],
                                              t_hin.ap()[:, it0 * P:(it0 + nb) * P])
                        ps = dnp.tile([P, DB * oc], F32, tag="ps")
                        for q in range(nb):
                            lhsT = (xin[:, q * P:(q + 1) * P] if l == 0 else
                                    hT[:, (it0 + q) * P:(it0 + q + 1) * P])
                            nc.tensor.matmul(ps[:, q * oc:(q + 1) * oc], lhsT=lhsT,
                                             rhs=W_sb[:], start=True, stop=True)
                        psv = ps[:, :nb * oc].rearrange("p (q o) -> p q o", o=oc)
                        als_red = None
                        for which in range(2):
                            a_bc = av_sb[l][which][:].unsqueeze(1) \
                                .to_broadcast([P, nb, oc])
                            tmp = dn.tile([P, DB * oc], F32, tag=f"tmp{which}")
                            nc.vector.tensor_tensor(
                                out=tmp[:, :nb * oc].rearrange("p (q o) -> p q o", o=oc),
                                in0=psv, in1=a_bc, op=mybir.AluOpType.mult)
                            red = dn.tile([P, DB * 8], F32, tag=f"red{which}")
                            nc.vector.tensor_reduce(
                                out=red[:, :nb * heads],
                                in_=tmp[:, :nb * oc].rearrange(
                                    "p (q h c) -> p q h c", h=heads, c=ch),
                                axis=mybir.AxisListType.X, op=mybir.AluOpType.add)
                            if which == 0:
                                als_red = red
                            else:
                                dv = alD[:, it0 * 8:(it0 + nb) * 8] \
                                    .rearrange("p (q e) -> p q e", e=8)[:, :, :heads]
                                nc.vector.tensor_copy(
                                    out=dv, in_=red[:, :nb * heads]
                                    .rearrange("p (q h) -> p q h", h=heads))
                        hc = dn.tile([P, DB * rl], F16, tag="hc")
                        hcv = hc[:, :nb * rl].rearrange("p (q r) -> p q r", r=rl)
                        if l < 2:
                            b_bc = bv_sb[l][:].unsqueeze(1).to_broadcast([P, nb, oc])
                            nc.vector.tensor_tensor(out=hcv[:, :, 0:oc], in0=psv,
                                                    in1=b_bc, op=mybir.AluOpType.add)
                        else:
                            nc.vector.tensor_copy(out=hcv[:, :, 0:oc], in_=psv)
                        nc.vector.tensor_copy(
                            out=hcv[:, :, oc:oc + heads],
                            in_=als_red[:, :nb * heads].rearrange("p (q h) -> p q h", h=heads))
                        if it0 + nb == n_chunks:
                            # last chunk is all pad rows: poison its al_s
                            nc.gpsimd.memset(
                                hc[:, (nb - 1) * rl + oc:(nb - 1) * rl + oc + heads],
                                PAD_ALS)
                        nc.sync.dma_start(
                            hcat_loc[l][:][it0 * P:(it0 + nb) * P, :]
                            .rearrange("(q p) r -> p q r", p=P), hcv)
                # ---------------- allgather ----------------
                nc.gpsimd.collective_compute(
                    "AllGather", mybir.AluOpType.bypass,
                    replica_groups=[list(range(NC))],
                    ins=[hcat_loc[l].opt()], outs=[hcat_full[l].opt()])
                # ---------------- edge phase ----------------
                relu = l < 2
                if True:
                    if l == 2:
                        ysum_ps = egp.tile([1, OC[2]], F32, tag="ysum")
                        n_mm = sum(nb for (_, _, nb, _) in pl.iters)
                        i_mm = 0
                    for (K, c0, nb, coff) in pl.iters:
                        ns = K * nb
                        idx_sb = eg.tile([P, BATCH_SLOTS], I32, tag="idx")
                        nc.sync.dma_start(idx_sb[:, :ns], t_idx.ap()[:, coff:coff + ns])
                        g = eg.tile([P, BATCH_SLOTS * rl], F16, tag="g")
                        for k in range(ns):
                            nc.gpsimd.indirect_dma_start(
                                out=g[:, k * rl:(k + 1) * rl], out_offset=None,
                                in_=hcat_full[l][:],
                                in_offset=bass.IndirectOffsetOnAxis(
                                    ap=idx_sb[:, k:k + 1], axis=0))
                        gv = g[:, :ns * rl].rearrange("p (s r) -> p s r", r=rl)
                        # logits = al_s + al_d ; lrelu ; exp
                        lg = eg1.tile([P, BATCH_SLOTS * 8], F16, tag="lg")
                        lgv = lg[:, :ns * heads]
                        al_d_bc = alD[:, c0 * 8:(c0 + nb) * 8] \
                            .rearrange("p (q e) -> p q e", e=8)[:, :, :heads] \
                            .unsqueeze(2).to_broadcast([P, nb, K, heads])
                        nc.vector.tensor_tensor(
                            out=lgv.rearrange("p (q k h) -> p q k h", k=K, h=heads),
                            in0=gv[:, :, oc:oc + heads]
                            .rearrange("p (q k) h -> p q k h", k=K),
                            in1=al_d_bc, op=mybir.AluOpType.add)
                        lg2 = eg1.tile([P, BATCH_SLOTS * 8], F16, tag="lg2")
                        nc.vector.tensor_scalar_mul(lg2[:, :ns * heads], lgv, NEG_SLOPE)
                        lg3 = eg1.tile([P, BATCH_SLOTS * 8], F16, tag="lg3")
                        nc.vector.tensor_tensor(out=lg3[:, :ns * heads], in0=lgv,
                                                in1=lg2[:, :ns * heads],
                                                op=mybir.AluOpType.max)
                        es = eg1.tile([P, BATCH_SLOTS * 8], F16, tag="es")
                        nc.scalar.activation(out=es[:, :ns * heads], in_=lg3[:, :ns * heads],
                                             func=mybir.ActivationFunctionType.Exp)
                        den = eg1.tile([P, MAXB * 8], F32, tag="den")
                        nc.vector.tensor_reduce(
                            out=den[:, :nb * heads],
                            in_=es[:, :ns * heads].rearrange(
                                "p (q k h) -> p q h k", k=K, h=heads),
                            axis=mybir.AxisListType.X, op=mybir.AluOpType.add)
                        den2 = eg1.tile([P, MAXB * 8], F32, tag="den2")
                        m_bc = mask_sb[:, c0:c0 + nb].unsqueeze(2) \
                            .to_broadcast([P, nb, heads])
                        nc.vector.tensor_tensor(
                            out=den2[:, :nb * heads].rearrange("p (q h) -> p q h", h=heads),
                            in0=den[:, :nb * heads].rearrange("p (q h) -> p q h", h=heads),
                            in1=m_bc, op=mybir.AluOpType.add)
                        inv = eg1.tile([P, MAXB * 8], F32, tag="inv")
                        nc.vector.reciprocal(out=inv[:, :nb * heads], in_=den2[:, :nb * heads])
                        # e16 = exp(logits) expanded over ch (one ACT op)
                        e16 = eg1.tile([P, BATCH_SLOTS * 128], F16, tag="e16")
                        e16v = e16[:, :ns * oc]
                        nc.scalar.activation(
                            out=e16v.rearrange("p (s h c) -> p s h c", h=heads, c=ch),
                            in_=lg3[:, :ns * heads].rearrange("p (s h) -> p s h", h=heads)
                            .unsqueeze(3).to_broadcast([P, ns, heads, ch]),
                            func=mybir.ActivationFunctionType.Exp)
                        mp = eg1.tile([P, BATCH_SLOTS * 128], F16, tag="mp")
                        nc.vector.tensor_tensor(
                            out=mp[:, :ns * oc].rearrange("p (s c) -> p s c", c=oc),
                            in0=gv[:, :, 0:oc],
                            in1=e16v.rearrange("p (s c) -> p s c", c=oc),
                            op=mybir.AluOpType.mult)
                        # fold-tree reduce over K -> msum fp32
                        scrA = eg1.tile([P, BATCH_SLOTS * 64], F16, tag="scrA")
                        scrB = eg1.tile([P, BATCH_SLOTS * 48], F16, tag="scrB")
                        cur, curk = mp, K
                        while curk > 1:
                            a_in = cur[:, :nb * curk * oc].rearrange(
                                "p (q k c) -> p q k c", k=curk, c=oc)
                            if curk % 2 == 1:
                                half = (curk + 1) // 2
                                pair = curk - half
                            else:
                                half, pair = curk // 2, curk // 2
                            dst_t = scrA if cur is not scrA else scrB
                            o_v = dst_t[:, :nb * half * oc].rearrange(
                                "p (q k c) -> p q k c", k=half, c=oc)
                            nc.vector.tensor_tensor(
                                out=o_v[:, :, 0:pair], in0=a_in[:, :, 0:pair],
                                in1=a_in[:, :, half:half + pair], op=mybir.AluOpType.add)
                            if half > pair:
                                nc.vector.tensor_copy(out=o_v[:, :, pair:half],
                                                      in_=a_in[:, :, pair:half])
                            cur, curk = dst_t, half
                        # normalize (+relu)
                        hout = eg.tile([P, MAXB * 128], F16, tag="hout")
                        inv_bc = inv[:, :nb * heads].rearrange(
                            "p (q h) -> p q h", h=heads).unsqueeze(3).to_broadcast([P, nb, heads, ch])
                        nc.vector.tensor_tensor(
                            out=hout[:, :nb * oc].rearrange(
                                "p (q h c) -> p q h c", h=heads, c=ch),
                            in0=cur[:, :nb * oc].rearrange(
                                "p (q h c) -> p q h c", h=heads, c=ch),
                            in1=inv_bc, op=mybir.AluOpType.mult)
                        if relu:
                            hr = eg.tile([P, MAXB * 128], F16, tag="hr")
                            nc.scalar.activation(out=hr[:, :nb * oc], in_=hout[:, :nb * oc],
                                                 func=mybir.ActivationFunctionType.Relu)
                            for q in range(nb):
                                tp = egp.tile([P, P], F16, tag="tp")
                                nc.tensor.transpose(out=tp[:], in_=hr[:, q * oc:(q + 1) * oc],
                                                    identity=ident[:])
                                nc.vector.tensor_copy(
                                    out=hT[:, (c0 + q) * P:(c0 + q + 1) * P], in_=tp[:])
                        else:
                            for q in range(nb):
                                nc.tensor.matmul(
                                    ysum_ps[:], lhsT=ones_col[:],
                                    rhs=hout[:, q * oc:(q + 1) * oc],
                                    start=(i_mm == 0), stop=(i_mm == n_mm - 1))
                                i_mm += 1
            if L < 2:
                nc.sync.dma_start(t_hout.ap(), hT[:])
            else:
                ysb = res.tile([1, OC[2]], F32, tag="ysb")
                nc.vector.tensor_copy(out=ysb[:], in_=ysum_ps[:])
                nc.sync.dma_start(t_y.ap(), ysb[:])
    nc.compile()
    return nc


# ----------------------------------------------------------------- entry point

N_NODES, N_EDGES = 100000, 1600000
_CACHE = {}


def _get_compiled(edge_index):
    key = hash(edge_index.tobytes())
    if key not in _CACHE:
        pl = make_plan(edge_index, N_NODES, ncores=8)
        ncs = [build_program(pl, layer=l, C0=128, H=(8, 8, 1), CH=(16, 16, 32),
                             ncores=8) for l in range(3)]
        _CACHE.clear()
        _CACHE[key] = (pl, ncs)
    return _CACHE[key]


def kernel(x, edge_index, W1, a_src1, a_dst1, b1, W2, a_src2, a_dst2, b2,
           W3, a_src3, a_dst3, b3):
    from concourse import bass_utils
    x = np.asarray(x, np.float32)
    edge_index = np.asarray(edge_index, np.int32)
    pl, ncs = _get_compiled(edge_index)
    layer_params = [(W1, a_src1, a_dst1, b1), (W2, a_src2, a_dst2, b2),
                    (W3, a_src3, a_dst3, None)]
    hins = x_slices(pl, x)
    for l in range(3):
        W, a_s, a_d, b = layer_params[l]
        in_maps = layer_inputs(pl, l, hins, W, a_s, a_d, b)
        res = bass_utils.run_bass_kernel_spmd(ncs[l], in_maps,
                                              core_ids=list(range(8)))
        if l < 2:
            hins = [res.results[c]["hout"] for c in range(8)]
        else:
            tot = np.sum([res.results[c]["y"] for c in range(8)], axis=0)
    return (tot / np.float32(N_NODES)
            + np.asarray(b3, np.float32)[None, :]).astype(np.float32)

